# revision 1
# baseline (speedup 1.0000x reference)
"""Trainium2 Bass kernel for nn_ContrastiveCRFLoss (self-contained).

Math: for each batch b and sample pairs (n, m) over 2048 gathered pixels:
    out[b,n,m] = -(C[b,n,m] * (W1*exp(-cd - gd[b]/(2*BETA)) + W2*exp(-cd/(2*GAMMA))))
where C = cluster Gram, cd = squared coord distance, gd = squared guidance
distance.

Device strategy (8 cores, grid-parallel over the n-rows):
  - Each core owns a 256-row block of the 2048x2048 pair grid, all 8 batches.
  - Three small-K fp16 matmuls per output tile, packed into PE row groups at
    partitions 0 / 32 / 64:
      group0 K=27: pC  = (-clusters)^T clusters            (negated Gram)
      group1 K=9 : p1  = full argument of the first exp    (augmented Gram)
      group2 K=12: p2  = full argument of the second exp   (coord-only, shared
                                                            across batches)
    The exp arguments are produced directly in PSUM by augmenting the Gram
    operands with ones/norm/log-weight rows, so no per-tile broadcast fixups
    are needed.  Norm rows are hi/lo split in fp16 and computed on the host
    from the fp16-snapped features, which keeps the Gram identity exact.
  - ACT: e = exp(PSUM) -> fp16 SBUF.  DVE: s = e1 + e2 (fp16 2x mode),
    out = pC * s (f32).  DMA out.
"""

import numpy as np

import concourse.bass as bass
import concourse.mybir as mybir
import concourse.bass_utils as bass_utils
from concourse.tile import TileContext
from concourse.vector_clock import ScopedClock

F16 = mybir.dt.float16
BF16 = mybir.dt.bfloat16
F32 = mybir.dt.float32

# problem constants (hardcoded per the task contract)
ALPHA, BETA, GAMMA = 0.5, 0.15, 25.0
W1, W2, SHIFT = 10.0, 3.0, 0.0
B, CG, CC, H = 8, 3, 27, 224
NS = 2048  # samples
NCORES = 8
MT = 2  # 128-row M-tiles per core
KC, K1, K2 = 27, 9, 12
NT = 4  # 512-wide matmul free-dim tiles per 2048 row

# ---------------------------------------------------------------------------
# Walrus in this image rejects >1 sync wait per instruction. Split the Tile
# tail-drain's waits and any multi-wait instruction into single-wait NOPs.
# ---------------------------------------------------------------------------
_MAXW = 1


def _split_drain_and_barrier(self, tick_clock, wait_clock):
    probe = self.nc.sync.nop(nofuse=True)
    wait_clock.add_sem_waits(probe.ins, ScopedClock({None: tick_clock.global_clock}))
    si = probe.ins.sync_info
    waits = list(si.on_wait)
    probe.ins.sync_info = mybir.SyncInfo(
        on_wait=waits[:_MAXW], on_update=list(si.on_update)
    )
    for i in range(_MAXW, len(waits), _MAXW):
        n2 = self.nc.sync.nop(nofuse=True)
        n2.ins.sync_info = mybir.SyncInfo(on_wait=waits[i : i + _MAXW], on_update=[])
    self.nc.sync.drain()
    self.nc.all_engine_barrier()
    popped = self.nc._tile_sem_poison_stack.pop()
    assert popped is self._sem_poison
    self.nc.clear_and_free_semaphores(list(self.sems.allocated().values()))
    self.nc.all_engine_barrier()


def _split_multiwait_insts(nc):
    n_split = 0
    for fn in nc.m.functions:
        for bb in fn.blocks:
            insts = list(bb.instructions)
            new_insts = []
            changed = False
            for inst in insts:
                si = inst.sync_info
                waits = list(si.on_wait) if si is not None else []
                if len(waits) > _MAXW:
                    n_split += 1
                    changed = True
                    n_extra = len(waits) - _MAXW
                    for i in range(0, n_extra, _MAXW):
                        nop = mybir.InstNoOp(
                            name=nc.get_next_instruction_name(),
                            engine=inst.engine,
                            bass_nofuse=True,
                            sync_info=mybir.SyncInfo(
                                on_wait=waits[i : i + _MAXW], on_update=[]
                            ),
                        )
                        new_insts.append(nop)
                    inst.sync_info = mybir.SyncInfo(
                        on_wait=waits[n_extra:], on_update=list(si.on_update)
                    )
                new_insts.append(inst)
            if changed:
                bb.instructions = new_insts
    return n_split


def _install_tile_patch():
    TileContext._drain_and_barrier = _split_drain_and_barrier


# ---------------------------------------------------------------------------
# Device program (identical on all cores; data differs per core)
# ---------------------------------------------------------------------------

def build_nc():
    _install_tile_patch()
    nc = bass.Bass()
    wc = nc.declare_dram_parameter("wc", [KC, MT * B * 128], F16, isOutput=False)
    a1 = nc.declare_dram_parameter("a1", [K1, MT * B * 128], F16, isOutput=False)
    a2 = nc.declare_dram_parameter("a2", [K2, MT * 128], F16, isOutput=False)
    rc = nc.declare_dram_parameter("rc", [KC, B * NS], F16, isOutput=False)
    r1 = nc.declare_dram_parameter("r1", [K1, B * NS], F16, isOutput=False)
    r2 = nc.declare_dram_parameter("r2", [K2, NS], F16, isOutput=False)
    out = nc.declare_dram_parameter("out", [B, MT, 128, NS], F32, isOutput=True)

    with TileContext(nc) as tc:
        with (
            tc.tile_pool(name="w", bufs=1) as wpool,
            tc.tile_pool(name="r", bufs=1) as rpool,
            tc.tile_pool(name="e2p", bufs=2) as e2pool,
            tc.tile_pool(name="sb", bufs=3) as sbpool,
            tc.tile_pool(name="ob", bufs=3) as opool,
            tc.tile_pool(name="ps", bufs=2, space="PSUM") as pspool,
        ):
            W = wpool.tile([128, MT * B * 128], F16)
            R = rpool.tile([128, B * NS], F16)
            nc.sync.dma_start(W[0:KC, :], wc[:])
            nc.sync.dma_start(W[32 : 32 + K1, :], a1[:])
            nc.sync.dma_start(W[64 : 64 + K2, 0 : MT * 128], a2[:])
            nc.sync.dma_start(R[0:KC, :], rc[:])
            nc.sync.dma_start(R[32 : 32 + K1, :], r1[:])
            nc.sync.dma_start(R[64 : 64 + K2, 0:NS], r2[:])

            HN = NS // 2  # 1024: half-row, 2 PSUM banks

            for m in range(MT):
                # batch-independent second-exp argument for this row block
                e2 = e2pool.tile([128, NS], F16, tag="e2")
                for h in range(2):
                    p2 = pspool.tile([128, HN], F32, tag="pa")
                    for j in range(2):
                        jj = h * 2 + j
                        nc.tensor.matmul(
                            p2[:, j * 512 : (j + 1) * 512],
                            W[64 : 64 + K2, m * 128 : (m + 1) * 128],
                            R[64 : 64 + K2, jj * 512 : (jj + 1) * 512],
                            start=True,
                            stop=True,
                            tile_position=(64, 0),
                        )
                    nc.scalar.activation(
                        e2[:, h * HN : (h + 1) * HN],
                        p2[:],
                        mybir.ActivationFunctionType.Exp,
                    )

                for b in range(B):
                    col = (b * MT + m) * 128
                    e1 = sbpool.tile([128, NS], F16, tag="e1")
                    s = sbpool.tile([128, NS], F16, tag="s")
                    o = opool.tile([128, NS], F32, tag="o")
                    p1h = [
                        pspool.tile([128, HN], F32, tag="pa", name=f"p1h{h}")
                        for h in range(2)
                    ]
                    pch = [
                        pspool.tile([128, HN], F32, tag="pb", name=f"pch{h}")
                        for h in range(2)
                    ]
                    # strictly alternate row groups q32/q0 so consecutive
                    # independent matmuls overlap in the PE array
                    for h in range(2):
                        for j in range(2):
                            jj = h * 2 + j
                            nc.tensor.matmul(
                                p1h[h][:, j * 512 : (j + 1) * 512],
                                W[32 : 32 + K1, col : col + 128],
                                R[
                                    32 : 32 + K1,
                                    b * NS + jj * 512 : b * NS + (jj + 1) * 512,
                                ],
                                start=True,
                                stop=True,
                                tile_position=(32, 0),
                            )
                            nc.tensor.matmul(
                                pch[h][:, j * 512 : (j + 1) * 512],
                                W[0:KC, col : col + 128],
                                R[
                                    0:KC,
                                    b * NS + jj * 512 : b * NS + (jj + 1) * 512,
                                ],
                                start=True,
                                stop=True,
                                tile_position=(0, 0),
                            )
                        nc.scalar.activation(
                            e1[:, h * HN : (h + 1) * HN],
                            p1h[h][:],
                            mybir.ActivationFunctionType.Exp,
                        )
                    # add split across DVE (fast) and GpSimd (slow) halves
                    nc.vector.tensor_add(s[:, 0:HN], e1[:, 0:HN], e2[:, 0:HN])
                    nc.gpsimd.tensor_add(s[:, HN:NS], e1[:, HN:NS], e2[:, HN:NS])
                    for h in range(2):
                        nc.vector.tensor_tensor(
                            o[:, h * HN : (h + 1) * HN],
                            pch[h][:],
                            s[:, h * HN : (h + 1) * HN],
                            mybir.AluOpType.mult,
                        )
                    nc.sync.dma_start(out[b, m], o[:])

    _split_multiwait_insts(nc)
    return nc


# ---------------------------------------------------------------------------
# Host-side input prep
# ---------------------------------------------------------------------------

def _f16(x):
    return np.asarray(x, dtype=np.float16)


def _hi_lo(x):
    """Split fp64 vector into two fp16 rows summing to ~x."""
    hi = _f16(x)
    lo = _f16(x - hi.astype(np.float64))
    return hi, lo


def prepare_inputs(guidance, clusters, coords):
    ci = np.asarray(coords[0], dtype=np.int64)
    cj = np.asarray(coords[1], dtype=np.int64)
    # gathers: [B, C, NS]
    sel_g = guidance[:, :, ci, cj].astype(np.float64)
    sel_c = clusters[:, :, ci, cj].astype(np.float32)

    # --- cluster Gram operands (fp16 snap) ---
    c16 = _f16(sel_c)  # [B, 27, NS] rhs
    wc_all = -c16  # lhsT (negated -> folds the leading minus)

    # --- first-exp argument operands ---
    # arg1 = -cd/(2a) - gd/(2beta) + ln(W1) ; 2a = 1
    u16 = _f16(sel_g / np.sqrt(2.0 * BETA))  # [B, 3, NS]
    xc16 = _f16((np.stack([ci, cj]) - 112.0))  # [2, NS] exact
    f1 = (u16.astype(np.float64) ** 2).sum(1) + (
        xc16.astype(np.float64) ** 2
    ).sum(0)  # [B, NS]
    a1_all = np.empty((B, K1, NS), np.float16)
    r1_all = np.empty((B, K1, NS), np.float16)
    ones = np.ones(NS, np.float16)
    for b in range(B):
        b1 = np.log(W1) - f1[b]
        b1h, b1l = _hi_lo(b1)
        f1h, f1l = _hi_lo(f1[b])
        a1_all[b, 0:3] = u16[b]
        a1_all[b, 3:5] = xc16
        a1_all[b, 5] = ones
        a1_all[b, 6] = ones
        a1_all[b, 7] = f1h
        a1_all[b, 8] = f1l
        r1_all[b, 0:3] = _f16(2.0 * u16[b].astype(np.float64))
        r1_all[b, 3:5] = _f16(2.0 * xc16.astype(np.float64))
        r1_all[b, 5] = b1h
        r1_all[b, 6] = b1l
        r1_all[b, 7] = -ones
        r1_all[b, 8] = -ones

    # --- second-exp argument operands (batch independent) ---
    v = (np.stack([ci, cj]) - 112.0) / np.sqrt(2.0 * GAMMA)  # [2, NS] fp64
    vh = _f16(v)
    vl = _f16(v - vh.astype(np.float64))
    vs = vh.astype(np.float64) + vl.astype(np.float64)  # snapped value
    f2 = (vs**2).sum(0)  # [NS]
    b2 = np.log(W2) - f2
    b2h, b2l = _hi_lo(b2)
    f2h, f2l = _hi_lo(f2)
    a2 = np.empty((K2, NS), np.float16)
    r2 = np.empty((K2, NS), np.float16)
    # cross products: (vh+vl)_n * 2*(vh+vl)_m  per dim
    a2[0:2] = vh
    a2[2:4] = vh
    a2[4:6] = vl
    a2[6:8] = vl
    r2[0:2] = _f16(2.0 * vh.astype(np.float64))
    r2[2:4] = _f16(2.0 * vl.astype(np.float64))
    r2[4:6] = _f16(2.0 * vh.astype(np.float64))
    r2[6:8] = _f16(2.0 * vl.astype(np.float64))
    a2[8] = ones
    a2[9] = ones
    a2[10] = f2h
    a2[11] = f2l
    r2[8] = b2h
    r2[9] = b2l
    r2[10] = -ones
    r2[11] = -ones

    # --- per-core input maps (core k owns n-rows [256k, 256k+256)) ---
    in_maps = []
    for k in range(NCORES):
        rows = slice(256 * k, 256 * k + 256)
        # A-side column layouts: [(b * MT + m) * 128] for wc/a1, [m * 128] for a2
        wc_k = wc_all[:, :, rows].transpose(1, 0, 2).reshape(KC, B * 256)
        a1_k = a1_all[:, :, rows].transpose(1, 0, 2).reshape(K1, B * 256)
        a2_k = np.ascontiguousarray(a2[:, rows])
        in_maps.append(
            {
                "wc": np.ascontiguousarray(wc_k),
                "a1": np.ascontiguousarray(a1_k),
                "a2": a2_k,
                "rc": np.ascontiguousarray(c16.transpose(1, 0, 2).reshape(KC, B * NS)),
                "r1": np.ascontiguousarray(
                    r1_all.transpose(1, 0, 2).reshape(K1, B * NS)
                ),
                "r2": r2,
            }
        )
    return in_maps


_NC_CACHE = {}


def _get_nc():
    if "nc" not in _NC_CACHE:
        _NC_CACHE["nc"] = build_nc()
    return _NC_CACHE["nc"]


def kernel(guidance, clusters, coords):
    guidance = np.asarray(guidance)
    clusters = np.asarray(clusters)
    coords = np.asarray(coords)
    in_maps = prepare_inputs(guidance, clusters, coords)
    nc = _get_nc()
    res = bass_utils.run_bass_kernel_spmd(nc, in_maps, list(range(NCORES)))
    # res.results[k]["out"]: [B, MT, 128, NS] -> rows 256k..256k+256 of [B, NS, NS]
    full = np.concatenate(
        [res.results[k]["out"].reshape(B, MT * 128, NS) for k in range(NCORES)],
        axis=1,
    )
    return full.astype(np.float32)



# revision 2
# speedup vs baseline: 1.4122x; 1.4122x over previous
"""Trainium2 Bass kernel for nn_ContrastiveCRFLoss (self-contained).

Math: for batch b and sample pairs (n, m) over 2048 gathered pixels:
    out[b,n,m] = -(C[b,n,m] * (W1*exp(-cd - gd[b]/(2*BETA)) + W2*exp(-cd/(2*GAMMA))))
where C = cluster Gram, cd = squared coord distance, gd = squared guidance
distance.  The output is SYMMETRIC in (n, m): C, cd, gd all are.

Device strategy (8 cores, upper-triangle only, fp16 output):
  - The 2048x2048 pair grid is tiled 16 row-tiles x 4 col-chunks (128x512).
    Core k owns row-tiles {k, 15-k}; for row-tile r only col-chunks
    j >= r//4 are computed (aligned-down staircase).  Every core gets
    exactly 5 (row-tile, chunk) blocks -> a single SPMD program; the
    per-core block list is baked into the operand packing (data-driven
    addressing), not the program.
  - Per block, per batch-pair, three small-K fp16 matmuls packed into PE
    row groups 0/32/64 (K=27 cluster Gram, K=9 first-exp argument, K=12
    second-exp argument; exp args produced directly in PSUM via augmented
    operands exactly as in the dense baseline).
  - ACT: e1/e2 = exp(PSUM) -> fp16.  DVE/GpSimd: s = e1+e2 (fp16 2x mode),
    o = pC*s -> fp16 (some tiles routed via an ACT PSUM->fp16 copy so the
    multiply runs in DVE 2x mode).  DMA out fp16.
  - Host: assemble blocks into the upper triangle, mirror to the lower
    triangle, upcast to f32.
"""

import numpy as np

import concourse.bass as bass
import concourse.mybir as mybir
import concourse.bass_utils as bass_utils
from concourse.tile import TileContext
from concourse.vector_clock import ScopedClock

F16 = mybir.dt.float16
F32 = mybir.dt.float32

# problem constants (hardcoded per the task contract)
ALPHA, BETA, GAMMA = 0.5, 0.15, 25.0
W1, W2, SHIFT = 10.0, 3.0, 0.0
B, CG, CC, H = 8, 3, 27, 224
NS = 2048  # samples
NCORES = 8
KC, K1, K2 = 27, 9, 12
NBLK = 5     # (row-tile, col-chunk) blocks per core
NPAIR = 4    # batch pairs
CW = 512     # chunk width

# routing tables over the 20 macro-tiles (block-major, pair-minor)
ADD_ON_GP = set(range(1, 20, 2))          # 10 tiles add on GpSimd
MULT_VIA_COPY = {2, 7, 12, 17}            # 4 tiles: ACT copy -> fp16 2x mult


def core_blocks(k):
    """Block list for core k: (row_tile, col_chunk) pairs, 5 entries."""
    out = []
    for r in (k, 15 - k):
        out.extend((r, j) for j in range(r // 4, 4))
    return out


# ---------------------------------------------------------------------------
# Walrus in this image rejects >1 sync wait per instruction. Split the Tile
# tail-drain's waits and any multi-wait instruction into single-wait NOPs.
# ---------------------------------------------------------------------------
_MAXW = 1


def _split_drain_and_barrier(self, tick_clock, wait_clock):
    probe = self.nc.sync.nop(nofuse=True)
    wait_clock.add_sem_waits(probe.ins, ScopedClock({None: tick_clock.global_clock}))
    si = probe.ins.sync_info
    waits = list(si.on_wait)
    probe.ins.sync_info = mybir.SyncInfo(
        on_wait=waits[:_MAXW], on_update=list(si.on_update)
    )
    for i in range(_MAXW, len(waits), _MAXW):
        n2 = self.nc.sync.nop(nofuse=True)
        n2.ins.sync_info = mybir.SyncInfo(on_wait=waits[i : i + _MAXW], on_update=[])
    self.nc.sync.drain()
    self.nc.all_engine_barrier()
    popped = self.nc._tile_sem_poison_stack.pop()
    assert popped is self._sem_poison
    self.nc.clear_and_free_semaphores(list(self.sems.allocated().values()))
    self.nc.all_engine_barrier()


def _split_multiwait_insts(nc):
    n_split = 0
    for fn in nc.m.functions:
        for bb in fn.blocks:
            insts = list(bb.instructions)
            new_insts = []
            changed = False
            for inst in insts:
                si = inst.sync_info
                waits = list(si.on_wait) if si is not None else []
                if len(waits) > _MAXW:
                    n_split += 1
                    changed = True
                    n_extra = len(waits) - _MAXW
                    for i in range(0, n_extra, _MAXW):
                        nop = mybir.InstNoOp(
                            name=nc.get_next_instruction_name(),
                            engine=inst.engine,
                            bass_nofuse=True,
                            sync_info=mybir.SyncInfo(
                                on_wait=waits[i : i + _MAXW], on_update=[]
                            ),
                        )
                        new_insts.append(nop)
                    inst.sync_info = mybir.SyncInfo(
                        on_wait=waits[n_extra:], on_update=list(si.on_update)
                    )
                new_insts.append(inst)
            if changed:
                bb.instructions = new_insts
    return n_split


def _install_tile_patch():
    TileContext._drain_and_barrier = _split_drain_and_barrier


# ---------------------------------------------------------------------------
# Device program (identical on all cores; data differs per core)
# ---------------------------------------------------------------------------

def build_nc():
    _install_tile_patch()
    nc = bass.Bass()
    wc = nc.declare_dram_parameter("wc", [KC, NBLK * B * 128], F16, isOutput=False)
    w1 = nc.declare_dram_parameter("w1", [K1, NBLK * B * 128], F16, isOutput=False)
    w2 = nc.declare_dram_parameter("w2", [K2, NBLK * 128], F16, isOutput=False)
    rc = nc.declare_dram_parameter("rc", [KC, NBLK * B * CW], F16, isOutput=False)
    r1 = nc.declare_dram_parameter("r1", [K1, NBLK * B * CW], F16, isOutput=False)
    r2 = nc.declare_dram_parameter("r2", [K2, NBLK * CW], F16, isOutput=False)
    out = nc.declare_dram_parameter(
        "out", [NPAIR, NBLK, 128, 2 * CW], F16, isOutput=True
    )

    MUL = mybir.AluOpType.mult
    EXP = mybir.ActivationFunctionType.Exp

    with TileContext(nc) as tc:
        with (
            tc.tile_pool(name="w", bufs=1) as wpool,
            tc.tile_pool(name="r", bufs=1) as rpool,
            tc.tile_pool(name="e2p", bufs=2) as e2pool,
            tc.tile_pool(name="sb", bufs=3) as sbpool,
            tc.tile_pool(name="c16", bufs=2) as c16pool,
            tc.tile_pool(name="ob", bufs=3) as opool,
            tc.tile_pool(name="psa", bufs=2, space="PSUM") as papool,
            tc.tile_pool(name="psb", bufs=2, space="PSUM") as pbpool,
        ):
            W = wpool.tile([128, NBLK * B * 128], F16)
            R = rpool.tile([128, NBLK * B * CW], F16)
            nc.sync.dma_start(W[0:KC, :], wc[:])
            nc.sync.dma_start(W[32 : 32 + K1, :], w1[:])
            nc.sync.dma_start(W[64 : 64 + K2, 0 : NBLK * 128], w2[:])
            # rhs loads split per block for a fast pipeline start
            for i in range(NBLK):
                cs, ce = i * B * CW, (i + 1) * B * CW
                nc.sync.dma_start(R[0:KC, cs:ce], rc[:, cs:ce])
                nc.sync.dma_start(R[32 : 32 + K1, cs:ce], r1[:, cs:ce])
                nc.sync.dma_start(
                    R[64 : 64 + K2, i * CW : (i + 1) * CW],
                    r2[:, i * CW : (i + 1) * CW],
                )

            t = 0
            for i in range(NBLK):
                # batch-shared second-exp argument for this block
                pt = papool.tile([128, 2 * CW], F32, tag="pa", name=f"p2_{i}")
                nc.tensor.matmul(
                    pt[:, 0:CW],
                    W[64 : 64 + K2, i * 128 : (i + 1) * 128],
                    R[64 : 64 + K2, i * CW : (i + 1) * CW],
                    start=True,
                    stop=True,
                    tile_position=(64, 0),
                )
                e2c = e2pool.tile([128, CW], F16, tag="e2")
                nc.scalar.activation(e2c[:], pt[:, 0:CW], EXP)

                for P in range(NPAIR):
                    p1 = papool.tile([128, 2 * CW], F32, tag="pa", name=f"p1_{t}")
                    pc = pbpool.tile([128, 2 * CW], F32, tag="pb", name=f"pc_{t}")
                    for h in range(2):
                        b = 2 * P + h
                        cw_ = (i * B + b) * 128
                        cr_ = (i * B + b) * CW
                        nc.tensor.matmul(
                            p1[:, h * CW : (h + 1) * CW],
                            W[32 : 32 + K1, cw_ : cw_ + 128],
                            R[32 : 32 + K1, cr_ : cr_ + CW],
                            start=True,
                            stop=True,
                            tile_position=(32, 0),
                        )
                        nc.tensor.matmul(
                            pc[:, h * CW : (h + 1) * CW],
                            W[0:KC, cw_ : cw_ + 128],
                            R[0:KC, cr_ : cr_ + CW],
                            start=True,
                            stop=True,
                            tile_position=(0, 0),
                        )
                    e1 = sbpool.tile([128, 2 * CW], F16, tag="e1")
                    s = sbpool.tile([128, 2 * CW], F16, tag="s")
                    o = opool.tile([128, 2 * CW], F16, tag="o")
                    nc.scalar.activation(e1[:], p1[:], EXP)
                    adder = nc.gpsimd if t in ADD_ON_GP else nc.vector
                    for h in range(2):
                        adder.tensor_add(
                            s[:, h * CW : (h + 1) * CW],
                            e1[:, h * CW : (h + 1) * CW],
                            e2c[:],
                        )
                    if t in MULT_VIA_COPY:
                        pc16 = c16pool.tile([128, 2 * CW], F16, tag="pc16")
                        nc.scalar.copy(pc16[:], pc[:])
                        nc.vector.tensor_tensor(o[:], pc16[:], s[:], MUL)
                    else:
                        nc.vector.tensor_tensor(o[:], pc[:], s[:], MUL)
                    nc.sync.dma_start(out[P, i], o[:])
                    t += 1

    _split_multiwait_insts(nc)
    return nc


# ---------------------------------------------------------------------------
# Host-side input prep
# ---------------------------------------------------------------------------

def _f16(x):
    return np.asarray(x, dtype=np.float16)


def _hi_lo(x):
    """Split fp64 vector into two fp16 rows summing to ~x."""
    hi = _f16(x)
    lo = _f16(x - hi.astype(np.float64))
    return hi, lo


def _build_operands(guidance, clusters, coords):
    """Full-length A-side / R-side operand stacks (as in the dense baseline)."""
    ci = np.asarray(coords[0], dtype=np.int64)
    cj = np.asarray(coords[1], dtype=np.int64)
    sel_g = guidance[:, :, ci, cj].astype(np.float64)  # [B, 3, NS]
    sel_c = clusters[:, :, ci, cj].astype(np.float32)  # [B, 27, NS]

    c16 = _f16(sel_c)  # [B, 27, NS]

    u16 = _f16(sel_g / np.sqrt(2.0 * BETA))  # [B, 3, NS]
    xc16 = _f16(np.stack([ci, cj]) - 112.0)  # [2, NS] exact
    f1 = (u16.astype(np.float64) ** 2).sum(1) + (
        xc16.astype(np.float64) ** 2
    ).sum(0)  # [B, NS]
    ones = np.ones(NS, np.float16)
    a1 = np.empty((B, K1, NS), np.float16)
    r1 = np.empty((B, K1, NS), np.float16)
    for b in range(B):
        b1h, b1l = _hi_lo(np.log(W1) - f1[b])
        f1h, f1l = _hi_lo(f1[b])
        a1[b, 0:3] = u16[b]
        a1[b, 3:5] = xc16
        a1[b, 5] = ones
        a1[b, 6] = ones
        a1[b, 7] = f1h
        a1[b, 8] = f1l
        r1[b, 0:3] = _f16(2.0 * u16[b].astype(np.float64))
        r1[b, 3:5] = _f16(2.0 * xc16.astype(np.float64))
        r1[b, 5] = b1h
        r1[b, 6] = b1l
        r1[b, 7] = -ones
        r1[b, 8] = -ones

    v = (np.stack([ci, cj]) - 112.0) / np.sqrt(2.0 * GAMMA)  # [2, NS]
    vh = _f16(v)
    vl = _f16(v - vh.astype(np.float64))
    vs = vh.astype(np.float64) + vl.astype(np.float64)
    f2 = (vs**2).sum(0)
    b2h, b2l = _hi_lo(np.log(W2) - f2)
    f2h, f2l = _hi_lo(f2)
    a2 = np.empty((K2, NS), np.float16)
    r2 = np.empty((K2, NS), np.float16)
    a2[0:2] = vh
    a2[2:4] = vh
    a2[4:6] = vl
    a2[6:8] = vl
    r2[0:2] = _f16(2.0 * vh.astype(np.float64))
    r2[2:4] = _f16(2.0 * vl.astype(np.float64))
    r2[4:6] = _f16(2.0 * vh.astype(np.float64))
    r2[6:8] = _f16(2.0 * vl.astype(np.float64))
    a2[8] = ones
    a2[9] = ones
    a2[10] = f2h
    a2[11] = f2l
    r2[8] = b2h
    r2[9] = b2l
    r2[10] = -ones
    r2[11] = -ones
    return c16, a1, r1, a2, r2


def prepare_inputs(guidance, clusters, coords):
    c16, a1, r1, a2, r2 = _build_operands(
        np.asarray(guidance), np.asarray(clusters), np.asarray(coords)
    )
    wc_all = -c16  # folds the leading minus into the Gram lhsT

    in_maps = []
    for k in range(NCORES):
        blocks = core_blocks(k)
        wc_k = np.empty((KC, NBLK * B * 128), np.float16)
        w1_k = np.empty((K1, NBLK * B * 128), np.float16)
        w2_k = np.empty((K2, NBLK * 128), np.float16)
        rc_k = np.empty((KC, NBLK * B * CW), np.float16)
        r1_k = np.empty((K1, NBLK * B * CW), np.float16)
        r2_k = np.empty((K2, NBLK * CW), np.float16)
        for i, (r, j) in enumerate(blocks):
            rows = slice(128 * r, 128 * r + 128)
            cols = slice(CW * j, CW * j + CW)
            w2_k[:, i * 128 : (i + 1) * 128] = a2[:, rows]
            r2_k[:, i * CW : (i + 1) * CW] = r2[:, cols]
            for b in range(B):
                cw_ = (i * B + b) * 128
                cr_ = (i * B + b) * CW
                wc_k[:, cw_ : cw_ + 128] = wc_all[b, :, rows]
                w1_k[:, cw_ : cw_ + 128] = a1[b, :, rows]
                rc_k[:, cr_ : cr_ + CW] = c16[b, :, cols]
                r1_k[:, cr_ : cr_ + CW] = r1[b, :, cols]
        in_maps.append(
            {"wc": wc_k, "w1": w1_k, "w2": w2_k, "rc": rc_k, "r1": r1_k, "r2": r2_k}
        )
    return in_maps


_NC_CACHE = {}


def _get_nc():
    if "nc" not in _NC_CACHE:
        _NC_CACHE["nc"] = build_nc()
    return _NC_CACHE["nc"]


_LOWER_MASK = None


def assemble(results):
    """Scatter per-core blocks into the upper triangle, mirror, upcast."""
    global _LOWER_MASK
    full = np.zeros((B, NS, NS), np.float32)
    for k in range(NCORES):
        o = results[k]["out"]  # [NPAIR, NBLK, 128, 2*CW] fp16
        for i, (r, j) in enumerate(core_blocks(k)):
            rows = slice(128 * r, 128 * r + 128)
            cols = slice(CW * j, CW * j + CW)
            for P in range(NPAIR):
                full[2 * P, rows, cols] = o[P, i, :, 0:CW]
                full[2 * P + 1, rows, cols] = o[P, i, :, CW : 2 * CW]
    if _LOWER_MASK is None:
        _LOWER_MASK = np.tri(NS, NS, -1, dtype=bool)
    fullT = np.swapaxes(full, 1, 2)
    full[:, _LOWER_MASK] = fullT[:, _LOWER_MASK]
    return full


def kernel(guidance, clusters, coords):
    guidance = np.asarray(guidance)
    clusters = np.asarray(clusters)
    coords = np.asarray(coords)
    in_maps = prepare_inputs(guidance, clusters, coords)
    nc = _get_nc()
    res = bass_utils.run_bass_kernel_spmd(nc, in_maps, list(range(NCORES)))
    return assemble(res.results)
